# revision 1
# baseline (speedup 1.0000x reference)
"""CARAFE exact-fp32 hybrid kernel.

Natural layout (channels on partitions). Per chunk of 1024 output pixels
(2 source rows x 4 output rows... 2 row-pairs), per tap:
  - PE: 6 selection-matmuls broadcast mask row t to all 128 partitions.
    Masks are split hi/mid/lo into three bf16 arrays (host-side); the three
    K=25 bf16 matmuls accumulate in fp32 PSUM, reconstructing the fp32 mask
    to ~2^-24 -- effectively exact.  out = sel_t.T @ mask_s
  - DVE: fp32 tensor_tensor multiply feat_window x mb -> tmp (or directly
    into an accumulator for the two chain-head taps).
  - adds: two independent accumulator chains so DVE and GPSIMD never wait on
    each other: acc_d (DVE chain) and acc_g (GPSIMD chain), combined at the
    end with one DVE add.  All adds are fp32.
Everything in the value path is fp32 (or exactly representable) -> ~1e-7.
"""

import numpy as np

N, C, H, W = 2, 128, 128, 128
K, S, R = 5, 2, 2
NT = K * K
HQ = 4
HPC = H // HQ  # 32 source rows per core
PROWS, PCOLS = HPC + 2 * R, W + 2 * R  # 36, 132
OROWS = 2 * HPC  # 64 output rows per core
NCORES = 8
NSPLIT = 3  # bf16 mask splits
GPS_TAPS = 19  # taps 1..GPS_TAPS accumulate on the second chain (tap 1 = head)
PE_TAPS = 5   # last PE_TAPS taps accumulate on PE via exact fp32 identity-matmuls
# GPSIMD adds measured 8x slower than DVE on HW (dispatch/join overhead) --
# both chains run on the DVE; two chains still help instruction independence.
USE_GPS = False

_prog_cache = {}


def _build_program(repeats=1):
    import concourse.bacc as bacc
    import concourse.mybir as mybir
    from concourse.tile import TileContext

    f32 = mybir.dt.float32
    bf16 = mybir.dt.bfloat16

    nc = bacc.Bacc(None, target_bir_lowering=False)
    fp = nc.dram_tensor("featp", [C, PROWS * PCOLS], f32, kind="ExternalInput")
    # three bf16 mask splits concatenated along the free dim (all operands
    # base-partition 0: accumulation groups with mixed base partitions fault)
    mk = nc.dram_tensor(
        "maskS", [NT, NSPLIT * OROWS * 2 * W], bf16, kind="ExternalInput"
    )
    sel = nc.dram_tensor("sel", [NT, NT * 128], bf16, kind="ExternalInput")
    identf = nc.dram_tensor("identf", [128, 128], f32, kind="ExternalInput")
    out = nc.dram_tensor("out", [C, OROWS * 2 * W], f32, kind="ExternalOutput")

    with TileContext(nc) as tc:
        with (
            tc.tile_pool(name="const", bufs=1) as cpool,
            tc.tile_pool(name="feat", bufs=1) as fpool,
            tc.tile_pool(name="mask", bufs=1) as mpool,
            tc.tile_pool(name="tmp", bufs=8) as tpool,
            tc.tile_pool(name="accs", bufs=3) as apool,
            tc.tile_pool(name="stage", bufs=3) as spool,
            tc.tile_pool(name="mb", bufs=2, space="PSUM") as mbpool,
            tc.tile_pool(name="accp", bufs=2, space="PSUM") as ppool,
        ):
            sel_sb = cpool.tile([NT, NT * 128], bf16)
            nc.sync.dma_start(out=sel_sb[:], in_=sel[:])
            identf_sb = cpool.tile([128, 128], f32)
            nc.sync.dma_start(out=identf_sb[:], in_=identf[:])
            feat_sb = fpool.tile([C, PROWS * PCOLS], f32)
            nc.sync.dma_start(out=feat_sb[:], in_=fp[:])
            mask_sb = mpool.tile([NT, NSPLIT * OROWS * 2 * W], bf16)
            nc.sync.dma_start(out=mask_sb[:], in_=mk[:])

            featv = feat_sb[:].rearrange("c (r w) -> c r w", w=PCOLS)
            # per split s: [25, s, blk, w, sh, sw]
            maskv = mask_sb[:].rearrange(
                "t (s blk sh w sw) -> t s blk w sh sw", s=NSPLIT, sh=2, w=W, sw=2
            )
            outv = out[:].rearrange("c (oh ow) -> c oh ow", ow=2 * W)

            import contextlib

            rep_ctx = tc.For_i(0, repeats, 1) if repeats > 1 else contextlib.nullcontext()
            with rep_ctx:
                _chunks(nc, tc, featv, maskv, outv, sel_sb, identf_sb, tpool, apool, spool, mbpool, ppool)
    nc.finalize()
    return nc


def _chunks(nc, tc, featv, maskv, outv, sel_sb, identf_sb, tpool, apool, spool, mbpool, ppool):
    import concourse.mybir as mybir

    f32 = mybir.dt.float32

    # tap 0 heads the DVE chain; tap 1 heads the GPSIMD chain; taps 2..GPS_TAPS
    # add on GPSIMD (early, so the GPS chain drains tmps as DVE produces them);
    # taps GPS_TAPS+1..24 add on DVE.
    g0 = 1  # head of gpsimd chain
    nchunks = HPC // 2
    for chunk in range(nchunks):
        hl = 2 * chunk
        acc_d = apool.tile([128, 1024], f32, tag="acc_d")
        acc_g = apool.tile([128, 1024], f32, tag="acc_g")
        acc_p = ppool.tile([128, 1024], f32)
        pe0 = NT - PE_TAPS  # taps pe0..24 accumulate on PE
        for t in range(NT):
            i, j = divmod(t, K)
            mb = mbpool.tile([128, 1024], f32)
            lhsT_sel = sel_sb[:, 128 * t : 128 * (t + 1)]
            for hh in range(2):
                for s in range(NSPLIT):
                    rhs = maskv[:, s, 2 * chunk + hh]
                    nc.tensor.matmul(
                        mb[:, 512 * hh : 512 * (hh + 1)],
                        lhsT=lhsT_sel,
                        rhs=rhs,
                        start=(s == 0),
                        stop=(s == NSPLIT - 1),
                    )
            fap = featv[:, hl + i : hl + i + 2, j : j + W]
            fap = fap[:, :, :, None].to_broadcast([C, 2, W, 4])
            if t == 0:
                dst = acc_d
            elif t == g0:
                dst = acc_g
            else:
                dst = tpool.tile([128, 1024], f32, tag="tmp")
            nc.vector.tensor_tensor(dst[:], fap, mb[:], mybir.AluOpType.mult)
            if t != 0 and t != g0:
                if t >= pe0:
                    # exact fp32 identity-matmul accumulate on the PE
                    for hh in range(2):
                        nc.tensor.matmul(
                            acc_p[:, 512 * hh : 512 * (hh + 1)],
                            lhsT=identf_sb[:],
                            rhs=dst[:, 512 * hh : 512 * (hh + 1)],
                            start=(t == pe0),
                            stop=(t == NT - 1),
                        )
                elif t <= GPS_TAPS:
                    (nc.gpsimd if USE_GPS else nc.vector).tensor_tensor(
                        acc_g[:], acc_g[:], dst[:], mybir.AluOpType.add
                    )
                else:
                    nc.vector.tensor_tensor(
                        acc_d[:], acc_d[:], dst[:], mybir.AluOpType.add
                    )
        # combine chains on DVE; ACT reorders (hh,w,sh,sw)->(oh,ow) into the
        # stage tile; contiguous DMA out
        nc.vector.tensor_tensor(acc_d[:], acc_d[:], acc_g[:], mybir.AluOpType.add)
        nc.vector.tensor_tensor(acc_d[:], acc_d[:], acc_p[:], mybir.AluOpType.add)
        stage = spool.tile([128, 1024], f32)
        av = acc_d[:].rearrange("c (hh w sh sw) -> c hh sh w sw", hh=2, w=W, sh=2, sw=2)
        for hh in range(2):
            nc.scalar.copy(stage[:, 512 * hh : 512 * (hh + 1)], av[:, hh])
        nc.sync.dma_start(
            out=outv[:, 4 * chunk : 4 * chunk + 4, :], in_=stage[:]
        )


def get_program(repeats=1):
    key = ("nc", repeats)
    if key not in _prog_cache:
        _prog_cache[key] = _build_program(repeats)
    return _prog_cache[key]


def make_in_maps(features, masks):
    features = np.asarray(features, dtype=np.float32)
    masks = np.asarray(masks, dtype=np.float32)

    def bf16(x):
        # round-to-nearest-even fp32 -> bf16, returned as fp32 values
        u = x.view(np.uint32)
        r = ((u >> 16) + ((u >> 15) & 1)).astype(np.uint32) << 16
        return r.view(np.float32)

    sel = np.zeros((NT, NT * 128), dtype=np.float32)
    for t in range(NT):
        sel[t, 128 * t : 128 * (t + 1)] = 1.0
    sel_b = _to_bf16_bytes(sel)

    in_maps = []
    for core in range(NCORES):
        n, q = divmod(core, HQ)
        h0 = HPC * q
        featp = np.zeros((C, PROWS, PCOLS), np.float32)
        lo = max(h0 - R, 0)
        hi = min(h0 + HPC + R, H)
        featp[:, (lo - (h0 - R)) : (hi - (h0 - R)), R : R + W] = features[n, :, lo:hi, :]
        m = masks[n, :, 2 * h0 : 2 * h0 + OROWS, :].reshape(NT, -1)
        m_hi = bf16(m)
        m_mid = bf16(m - m_hi)
        m_lo = bf16(m - m_hi - m_mid)
        maskS = np.concatenate([m_hi, m_mid, m_lo], axis=1)  # [25, 3*16384]
        in_maps.append(
            {
                "featp": featp.reshape(C, -1),
                "maskS": _to_bf16_bytes(maskS),
                "sel": sel_b,
                "identf": np.eye(128, dtype=np.float32),
            }
        )
    return in_maps


def _to_bf16_bytes(x32):
    """fp32 array whose values are bf16-representable -> ml_dtypes/np bf16 view."""
    import ml_dtypes

    return x32.astype(ml_dtypes.bfloat16)


def gather_output(results):
    out = np.empty((N, C, 2 * H, 2 * W), np.float32)
    for core in range(NCORES):
        n, q = divmod(core, HQ)
        oh0 = 2 * HPC * q
        out[n, :, oh0 : oh0 + OROWS, :] = results[core]["out"].reshape(C, OROWS, 2 * W)
    return out


def kernel(features, masks):
    from concourse.bass_utils import run_bass_kernel_spmd

    nc = get_program()
    in_maps = make_in_maps(features, masks)
    res = run_bass_kernel_spmd(nc, in_maps, core_ids=list(range(NCORES)))
    return gather_output(res.results)



# revision 2
# speedup vs baseline: 61.8054x; 61.8054x over previous
"""CARAFE banded-matmul kernel v3: row-quad packing, PSUM-bank-aligned blocks.

Same math as v2 (contraction (r in 8 rows) x (iwl in 16 window) = 128
partitions; one matmul covers 4 source rows x 2 output rows x 2Q cols), but
column blocks are [Q=12, Q=12, Q=8] per 512-float PSUM bank so no matmul
crosses a bank boundary: 4 banks x 3 classes = 12 matmuls per quad, 96/iter.

psum per quad [128, 2048] = (bank, class, d, o, w); reorder copies to stage
(d, o, ow) split ACT (o=0) / DVE (o=1); output bf16, one DMA per quad.
"""

import numpy as np

N, C, H, W = 2, 128, 128, 128
K, S, R = 5, 2, 2
NCORES = 8
HQ = 4
HPC = H // HQ      # 32 source rows per core
OROWS = 2 * HPC
D = 4              # source rows per quad
NQ = HPC // D      # 8 quads
NBANK = 4
QCL = [12, 12, 8]                      # class Q sizes within a bank (sum 32)
QB = QCL * NBANK
Q0 = [32 * bk + off for bk in range(NBANK) for off in (0, 12, 24)]
NB = len(QB)                           # 12 blocks per quad
XOFF = [0]
for q in QB:
    XOFF.append(XOFF[-1] + 4 * D * q)  # [0,192,384,512,704,896,1024,...]
SEG = XOFF[-1]                         # 2048 psum/B cols per quad
MQ = 1               # quads per output DMA
NR, NWIN = D + 4, 16                   # 8 rows x 16 window = 128 partitions
FCOLS = NQ * NB * C
BCOLS = NQ * SEG

_prog_cache = {}


def _build_body(nc, tc, featv, bv, outv, spool, ppool, f32, bf16):
    # class-major matmul order (A of all banks, then B, then C) so the
    # class-A reorder copies can start after 4 matmuls instead of 10
    border = [bk * 3 + cl for cl in range(3) for bk in range(NBANK)]
    for qp in range(NQ // MQ):
        st = spool.tile([128, MQ * SEG], bf16)
        for sub in range(MQ):
            q = MQ * qp + sub
            ps = ppool.tile([128, SEG], f32)
            for b in border:
                ncols = 4 * D * QB[b]
                nc.tensor.matmul(
                    ps[:, XOFF[b] : XOFF[b] + ncols],
                    lhsT=featv[:, q, b],
                    rhs=bv[:, q, XOFF[b] : XOFF[b] + ncols],
                    start=True,
                    stop=True,
                )
            # reorder psum (bank, cls, do, w) -> stage (do, bank, cls, w) with
            # (d,o) fused into one uniform dim: 3 copies per quad, 3-dim APs
            psv = ps[:].rearrange("c (bank x) -> c bank x", bank=NBANK)
            psab = psv[:, :, 0:384].rearrange(
                "c bank (sb do w) -> c sb do bank w", sb=2, do=2 * D
            )
            psc = psv[:, :, 384:512].rearrange(
                "c bank (do w) -> c do bank w", do=2 * D
            )
            stq = st[:, sub * SEG : (sub + 1) * SEG].rearrange(
                "c (do bank x) -> c do bank x", do=2 * D, bank=NBANK
            )
            nc.scalar.copy(stq[:, :, :, 0:24], psab[:, 0])
            nc.vector.tensor_copy(stq[:, :, :, 24:48], psab[:, 1])
            nc.scalar.copy(stq[:, :, :, 48:64], psc)
        nc.sync.dma_start(
            out=outv[:, 8 * MQ * qp : 8 * MQ * (qp + 1), :],
            in_=st[:].rearrange("c (oh ow) -> c oh ow", ow=2 * W),
        )


def _build_program(repeats=1):
    import concourse.bacc as bacc
    import concourse.mybir as mybir
    from concourse.tile import TileContext

    f32 = mybir.dt.float32
    bf16 = mybir.dt.bfloat16

    nc = bacc.Bacc(None, target_bir_lowering=False)
    ft = nc.dram_tensor("featR8", [128, FCOLS], bf16, kind="ExternalInput")
    bm = nc.dram_tensor("Bm", [128, BCOLS], bf16, kind="ExternalInput")
    out = nc.dram_tensor("out", [C, OROWS * 2 * W], bf16, kind="ExternalOutput")

    with TileContext(nc) as tc:
        with (
            tc.tile_pool(name="feat", bufs=1) as fpool,
            tc.tile_pool(name="mask", bufs=1) as mpool,
            tc.tile_pool(name="stage", bufs=6) as spool,
            tc.tile_pool(name="acc", bufs=2, space="PSUM") as ppool,
        ):
            ft_sb = fpool.tile([128, FCOLS], bf16)
            nc.sync.dma_start(out=ft_sb[:], in_=ft[:])
            bm_sb = mpool.tile([128, BCOLS], bf16)
            nc.sync.dma_start(out=bm_sb[:], in_=bm[:])

            featv = ft_sb[:].rearrange("p (q b c) -> p q b c", q=NQ, b=NB)
            bv = bm_sb[:].rearrange("p (q x) -> p q x", q=NQ)
            outv = out[:].rearrange("c (oh ow) -> c oh ow", ow=2 * W)

            import contextlib

            rep_ctx = tc.For_i(0, repeats, 1) if repeats > 1 else contextlib.nullcontext()
            with rep_ctx:
                _build_body(nc, tc, featv, bv, outv, spool, ppool, f32, bf16)
    nc.finalize()
    return nc


def get_program(repeats=1):
    key = ("carafe_pe", repeats)
    if key not in _prog_cache:
        _prog_cache[key] = _build_program(repeats)
    return _prog_cache[key]


def make_in_maps(features, masks):
    import ml_dtypes

    bf = ml_dtypes.bfloat16
    features = np.asarray(features, np.float32)
    masks = np.asarray(masks, np.float32)
    fb = features.astype(bf)
    mb = masks.astype(bf)

    fpad = np.zeros((N, C, H + 4, 140), bf)
    fpad[:, :, 2 : 2 + H, 2 : 2 + W] = fb

    rr = np.arange(NR)
    ww = np.arange(NWIN)
    qq = np.arange(NQ)
    q0a = np.array(Q0)

    in_maps = []
    for core in range(NCORES):
        n, qh = divmod(core, HQ)
        h0 = HPC * qh

        rows = h0 + D * qq[:, None, None, None] + rr[None, None, :, None]
        cols = q0a[None, :, None, None] + ww[None, None, None, :]
        A = fpad[n][:, rows, cols]                    # [C, NQ, NB, NR, NWIN]
        ft8 = A.transpose(3, 4, 1, 2, 0).reshape(128, NQ, NB, C)

        Bm = np.zeros((128, NQ, SEG), bf)
        for b in range(NB):
            qn = QB[b]
            owl = np.arange(2 * qn)
            j = ww[:, None] - (owl[None, :] // 2)
            dd = np.arange(D)
            i = rr[:, None] - dd[None, :]
            valid = ((j >= 0) & (j < 5))[None, :, None, None, None, :] & (
                (i >= 0) & (i < 5)
            )[:, None, None, :, None, None]
            t = (
                5 * np.clip(i, 0, 4)[:, None, None, :, None, None]
                + np.clip(j, 0, 4)[None, :, None, None, None, :]
            )
            oh = (
                2 * (h0 + D * qq[None, None, :, None, None, None]
                     + dd[None, None, None, :, None, None])
                + np.arange(2)[None, None, None, None, :, None]
            )
            ow = (2 * Q0[b] + owl)[None, None, None, None, None, :]
            vals = mb[n][t, oh, ow]                   # [8,16,8,4,2,2q]
            vals = np.where(valid, vals, bf(0))
            Bm[:, :, XOFF[b] : XOFF[b] + 4 * D * qn] = vals.reshape(
                128, NQ, 4 * D * qn
            )

        in_maps.append(
            {
                "featR8": np.ascontiguousarray(ft8.reshape(128, FCOLS)),
                "Bm": np.ascontiguousarray(Bm.reshape(128, BCOLS)),
            }
        )
    return in_maps


def gather_output(results):
    out = np.empty((N, C, 2 * H, 2 * W), np.float32)
    for core in range(NCORES):
        n, qh = divmod(core, HQ)
        oh0 = OROWS * qh
        out[n, :, oh0 : oh0 + OROWS, :] = (
            results[core]["out"].astype(np.float32).reshape(C, OROWS, 2 * W)
        )
    return out


def kernel(features, masks):
    from concourse.bass_utils import run_bass_kernel_spmd

    nc = get_program()
    in_maps = make_in_maps(features, masks)
    res = run_bass_kernel_spmd(nc, in_maps, core_ids=list(range(NCORES)))
    return gather_output(res.results)
